# revision 22
# baseline (speedup 1.0000x reference)
"""Single-head causal self-attention on 8 Trainium2 NeuronCores.

Reference computation (per batch b):
    k = x @ Wk.T ; q = x @ Wq.T ; v = x @ Wv.T
    wei = softmax(mask(q @ k.T / sqrt(H)))
    out = wei @ v

Strategy (v6):
  - Data parallel: shard B=256 across 8 cores (32 batches each), replicate
    weights. No cross-core communication.
  - Host-side weight preprocessing (O(C^2), negligible): G = Wq.T @ Wk * scale
    and WvT = Wv.T computed in numpy, so q @ k.T becomes x G x.T on chip and
    no weight transposes are needed. An AUX constant ([I | mask-bias | ones])
    is also shipped.
  - x is shipped bf16; x^T tiles come from the DMA XBAR transpose straight
    from HBM, one instruction per batch pair ([512,384] -> [128,3,512]).
  - All matmuls bf16 x bf16 -> fp32 PSUM. The tensor engine p-states up to
    ~0.42 ns/column once streaming continuously, so the whole schedule is
    built to never stall it: software-pipelined z2 two batches ahead,
    ST(b+1) issued before out(b), double-buffered score PSUM.
  - Causal mask is folded into the score accumulation as a bias matmul
    (I^T @ [R|0], R = -50 upper triangle) so exp needs no follow-up mask op;
    the s-hi/t-lo block is never computed at all.
  - Softmax denominator: ones columns are matmul-initialized in the V PSUM
    accumulation; the attention matmul then yields r[t] alongside out.
  - Output stored bf16 (host upcasts); normalization = vector reciprocal +
    scalar-engine scaled copy.
"""

import numpy as np
import ml_dtypes

import concourse.bass as bass
import concourse.mybir as mybir
from concourse import bacc
import concourse.tile as tile
from concourse.bass_utils import run_bass_kernel_spmd

B, T, C, H = 256, 256, 384, 384
NCORES = 8
NB = B // NCORES  # batches per core
P = 128
CC = C // P  # 3 chunks of the embedding dim
TC = T // P  # 2 chunks of the sequence dim
SCALE = float(H) ** -0.5
F32 = mybir.dt.float32
BF16 = mybir.dt.bfloat16
HP = H + 8  # v augmented with 8 ones columns (16B-aligned in bf16)
T2 = 2 * T  # 512: per-pair time span
NEG = -50.0  # causal mask bias (scores are O(10) at most)


def build_bass(nb: int = NB):
    assert nb % 2 == 0
    nc = bacc.Bacc(
        "TRN2",
        target_bir_lowering=False,
        debug=False,
        enable_asserts=False,
        num_devices=NCORES,
    )
    # x viewed flat as [(nb*T), C] so a batch pair is a 2D slice [512, C]
    x_d = nc.dram_tensor("x", [nb * T, C], BF16, kind="ExternalInput").ap()
    g_d = nc.dram_tensor("G", [C, C], BF16, kind="ExternalInput").ap()
    wvt_d = nc.dram_tensor("WvT", [C, H], BF16, kind="ExternalInput").ap()
    # AUX = [I(128) | R|0 (256) | R (128) | ones (8)], R = NEG upper triangle
    aux_d = nc.dram_tensor("AUX", [P, 520], BF16, kind="ExternalInput").ap()
    # output ships UNNORMALIZED with the r column appended ([H] sums + ones
    # sums); the host divides. This keeps reciprocal/normalize off the chip
    # so PSUM attention banks drain through a dependency-free copy.
    out_d = nc.dram_tensor("out", [nb * T, HP], BF16, kind="ExternalOutput").ap()

    with tile.TileContext(nc) as tc:
        with (
            tc.tile_pool(name="const", bufs=1) as cpool,
            tc.tile_pool(name="sb", bufs=3) as sb,
            tc.tile_pool(name="ob", bufs=8) as obp,
            tc.tile_pool(name="ps", bufs=1, space="PSUM") as psp,
        ):
            # G tiles [c1 partition chunk, c2 free] and WvT tiles [c, h free]
            g_s, wvT_s = [], []
            for cc_ in range(CC):
                g_t = cpool.tile([P, C], BF16, name=f"g{cc_}")
                nc.sync.dma_start(g_t, g_d[cc_ * P : (cc_ + 1) * P, :])
                g_s.append(g_t)
                w_t = cpool.tile([P, H], BF16, name=f"wvT{cc_}")
                nc.sync.dma_start(w_t, wvt_d[cc_ * P : (cc_ + 1) * P, :])
                wvT_s.append(w_t)
            aux = cpool.tile([P, 520], BF16, name="aux")
            nc.sync.dma_start(aux, aux_d)
            IDN = aux[:, 0:P]          # identity
            RB0 = aux[:, P : P + T]    # [R | 0] for the s-lo score group
            RB1 = aux[:, P + T : P + T + P]  # R for the s-hi score group
            ONES8 = aux[:, 512:520]

            n_pairs = nb // 2
            xTp_tiles = {}

            def emit_transpose(p):
                if p >= n_pairs or p in xTp_tiles:
                    return
                # xTp[c, cc, u] = x[pair, u, cc*128+c], u in [0,512) spanning
                # both batches of the pair; one XBAR transpose per pair.
                xTp = sb.tile([P, CC, T2], BF16, name="xTp", tag="xTp")
                nc.sync.dma_start(
                    xTp, x_d[p * T2 : (p + 1) * T2, :], transpose=True
                )
                xTp_tiles[p] = xTp

            zt_tiles = {}

            def emit_z2(b):
                # z2 = G^T x^T for batch b: [C, 256] (c2 chunks 0,1 in pzA,
                # chunk 2 in pzB). Consumes xTp of the pair b//2.
                if b >= nb:
                    return
                xTp = xTp_tiles[b // 2]
                off = (b & 1) * T
                pzA = psp.tile([P, 512], F32, name="pzA", tag="pzA")
                pzB = psp.tile([P, 512], F32, name="pzB", tag="pzB")[:, :T]
                for c2 in range(CC):
                    dst = pzA[:, (c2 * T) : ((c2 + 1) * T)] if c2 < 2 else pzB
                    for c1 in range(CC):
                        nc.tensor.matmul(
                            dst,
                            lhsT=g_s[c1][:, c2 * P : (c2 + 1) * P],
                            rhs=xTp[:, c1, off : off + T],
                            start=(c1 == 0),
                            stop=(c1 == CC - 1),
                        )
                zt = sb.tile([P, CC * T], BF16, name="zt", tag="zt")
                nc.vector.tensor_copy(zt[:, 0:512], pzA)
                nc.vector.tensor_copy(zt[:, 512:768], pzB)
                zt_tiles[b] = zt

            def emit_st(b):
                # ST[s, t] packed [128, 384]: cols 0:256 = (s-lo, t full),
                # cols 256:384 = (s-hi, t-hi); s-hi/t-lo never computed.
                # Causal bias NEG is matmul-accumulated so exp masks itself.
                xTp = xTp_tiles[b // 2]
                zt = zt_tiles.pop(b)
                off = (b & 1) * T
                pst = psp.tile([P, 512], F32, name="pst", tag="pst", bufs=2)[
                    :, : T + P
                ]
                nc.tensor.matmul(
                    pst[:, 0:T], lhsT=IDN, rhs=RB0, start=True, stop=False
                )
                for cc_ in range(CC):
                    nc.tensor.matmul(
                        pst[:, 0:T],
                        lhsT=xTp[:, cc_, off : off + P],
                        rhs=zt[:, cc_ * T : (cc_ + 1) * T],
                        start=False,
                        stop=(cc_ == CC - 1),
                    )
                nc.tensor.matmul(
                    pst[:, T : T + P], lhsT=IDN, rhs=RB1, start=True, stop=False
                )
                for cc_ in range(CC):
                    nc.tensor.matmul(
                        pst[:, T : T + P],
                        lhsT=xTp[:, cc_, off + P : off + T],
                        rhs=zt[:, cc_ * T + P : (cc_ + 1) * T],
                        start=False,
                        stop=(cc_ == CC - 1),
                    )
                et = sb.tile([P, T + P], BF16, name="et", tag="et")
                nc.scalar.activation(et, pst, mybir.ActivationFunctionType.Exp)
                return et

            def emit_v(b):
                # v_aug[sc] = [x[b] @ Wv.T | 1]  ([128, H+8] bf16); the ones
                # columns are matmul-initialized (I^T @ ones8) in PSUM.
                xTp = xTp_tiles[b // 2]
                off = (b & 1) * T
                vau = []
                for sc in range(TC):
                    pv = psp.tile([P, 512], F32, name="pv", tag="pv", bufs=2)
                    for cc_ in range(CC):
                        nc.tensor.matmul(
                            pv[:, :H],
                            lhsT=xTp[:, cc_, off + sc * P : off + (sc + 1) * P],
                            rhs=wvT_s[cc_],
                            start=(cc_ == 0),
                            stop=(cc_ == CC - 1),
                        )
                    vt = sb.tile([P, HP], BF16, name=f"v{sc}", tag=f"v{sc}")
                    nc.vector.tensor_copy(vt[:, :H], pv[:, :H])
                    nc.gpsimd.memset(vt[:, H:HP], 1.0)
                    vau.append(vt)
                return vau

            def emit_out_mm(b, et, vau):
                # out_un[t, h] = sum_s est[s, t] * v_aug[s, h]  (tensor only)
                pos = []
                for tcc in range(TC):
                    po = psp.tile([P, 512], F32, name="po", tag="po", bufs=2)[
                        :, :HP
                    ]
                    if tcc == 0:
                        nc.tensor.matmul(
                            po, lhsT=et[:, 0:P], rhs=vau[0], start=True,
                            stop=True,
                        )
                    else:
                        nc.tensor.matmul(
                            po, lhsT=et[:, P:T], rhs=vau[0], start=True,
                            stop=False,
                        )
                        nc.tensor.matmul(
                            po, lhsT=et[:, T : T + P], rhs=vau[1], start=False,
                            stop=True,
                        )
                    pos.append(po)
                return pos

            def emit_norm(b, pos):
                # drain + store (unnormalized, r column included); deferred so
                # the scalar queue's next exp is enqueued before these copies
                for tcc in range(TC):
                    po = pos[tcc]
                    ot = obp.tile([P, HP], BF16, name="ot", tag="ot")
                    nc.scalar.copy(ot, po)
                    nc.sync.dma_start(
                        out_d[b * T + tcc * P : b * T + (tcc + 1) * P, :], ot
                    )

            # prologue: fill the pipeline
            emit_transpose(0)
            emit_transpose(1)
            emit_z2(0)
            emit_z2(1)

            pending = None
            for pr in range(n_pairs):
                b0, b1 = 2 * pr, 2 * pr + 1
                et0 = emit_st(b0)
                if pending is not None:
                    emit_norm(*pending)
                    pending = None
                vau0 = emit_v(b0)
                et1 = emit_st(b1)
                pos0 = emit_out_mm(b0, et0, vau0)
                emit_norm(b0, pos0)
                vau1 = emit_v(b1)
                emit_transpose(pr + 2)
                emit_z2(b0 + 2)
                emit_z2(b1 + 2)
                pos1 = emit_out_mm(b1, et1, vau1)
                pending = (b1, pos1)
            emit_norm(*pending)

    nc.compile()
    return nc


_NC_CACHE = {}


def _get_nc(nb: int):
    if nb not in _NC_CACHE:
        _NC_CACHE[nb] = build_bass(nb)
    return _NC_CACHE[nb]


def _make_aux():
    aux = np.zeros((P, 520), dtype=np.float32)
    aux[:, 0:P] = np.eye(P, dtype=np.float32)  # identity
    r = np.where(
        np.arange(P)[None, :] < np.arange(P)[:, None], NEG, 0.0
    ).astype(np.float32)  # R[s, j] = NEG where j < s
    aux[:, P : 2 * P] = r  # [R | 0] first half
    aux[:, P + T : P + T + P] = r
    aux[:, 512:520] = 1.0
    return aux.astype(ml_dtypes.bfloat16)


def prep_in_maps(x, Wk, Wq, Wv):
    """Host-side shard + weight-only preprocessing -> per-core input maps."""
    x = np.asarray(x, dtype=np.float32)
    Wk = np.asarray(Wk, dtype=np.float32)
    Wq = np.asarray(Wq, dtype=np.float32)
    Wv = np.asarray(Wv, dtype=np.float32)
    G = np.ascontiguousarray((Wq.T @ Wk) * SCALE).astype(ml_dtypes.bfloat16)
    WvT = np.ascontiguousarray(Wv.T).astype(ml_dtypes.bfloat16)
    aux = _make_aux()
    nb = x.shape[0] // NCORES
    xb = np.ascontiguousarray(x).astype(ml_dtypes.bfloat16)
    xb = xb.reshape(NCORES, nb * x.shape[1], x.shape[2])
    return nb, [
        {"x": xb[i], "G": G, "WvT": WvT, "AUX": aux} for i in range(NCORES)
    ]


def kernel(x: np.ndarray, Wk: np.ndarray, Wq: np.ndarray, Wv: np.ndarray, **_):
    nb, in_maps = prep_in_maps(x, Wk, Wq, Wv)
    nc = _get_nc(nb)
    res = run_bass_kernel_spmd(nc, in_maps, core_ids=list(range(NCORES)))
    outs = []
    for r in res.results:
        oa = np.asarray(r["out"]).astype(np.float32).reshape(nb, T, HP)
        outs.append(oa[:, :, :H] / oa[:, :, H : H + 1])
    return np.concatenate(outs, axis=0)


if __name__ == "__main__":
    rng = np.random.default_rng(0)
    x = rng.standard_normal((B, T, C), dtype=np.float32)
    s = 1.0 / np.sqrt(C)
    Wk = rng.standard_normal((H, C), dtype=np.float32) * s
    Wq = rng.standard_normal((H, C), dtype=np.float32) * s
    Wv = rng.standard_normal((H, C), dtype=np.float32) * s
    out = kernel(x=x, Wk=Wk, Wq=Wq, Wv=Wv)
    print(out.shape, out.dtype)
